# revision 17
# baseline (speedup 1.0000x reference)
"""Trainium2 Bass kernel for nn_MemoryUnit (cosine-sim memory read with sparse
softmax shrinkage), data-parallel over 8 NeuronCores.

Per core (batch shard of 1024 rows), single fused pipeline:

  prologue : load x tiles (f32), row-square sums, fp16 copy, DMA-xbar
             transpose -> xT resident [f,b]; batched sqrt/recip -> invz
             (folded into the logit evict, so x casts need no norms).
  fused A+B1 (m-chunk outer, 512 cols = 4 mem row-tiles per chunk, mch
             triple-buffered, chunk preps emitted 3 iterations ahead):
             stream mem chunk, batched row norms, normalized fp16 copy,
             transpose -> mhatT chunk.  Per chunk: 8 bt x 16 k matmuls
             accumulate logits[bt, chunk] in 8 psum banks; evict = DVE
             psum*invz -> l16 (fp16 logit store; logits ~0 near the mask
             threshold so fp16 loses nothing there).
  threshold: per bt (pipelined under the last chunk's matmuls):
             e16 = exp(l16) in two halves with accum -> S; u = thr*S - 1;
             lnT = u(1-u/2); v16 = (l16 > lnT)*e16 with accum -> vsum;
             invV = 1/vsum; v transposed (2 half transposes) -> vT [m,b].
  B2       : out[b,f] = sum_m vT[m,b] * mem[m,f]; mem re-streamed from HBM
             with casting gpsimd DMA (fp32->fp16, no bounce buffer),
             fc-outer (4 waves of 512 f-cols) x 32 m k-tiles x 8 bt;
             evict scaled by invV (softmax S cancels algebraically).

Threshold identity: relu(w-t)*w/(|w-t|+1e-12) == w * 1{w>t} to ~1e-7 rel,
w = e/S, so mask is e > t*S <=> logit > ln(t*S); final L1 norm reduces to
division by sum(v).
"""
import sys

sys.path.insert(0, "/opt/trn_rl_repo")

import numpy as np

N_CORES = 8
B_FULL = 8192
B = B_FULL // N_CORES    # 1024 batch rows per core
M = 4000                 # memory rows
MP = 4096                # padded memory rows (transpose granularity)
F = 2048                 # features
P = 128

_CACHE = {}


def build_nc(B=B, M=M, MP=MP, F=F):
    import concourse.bacc as bacc
    import concourse.mybir as mybir
    import concourse.tile as tile

    fp32 = mybir.dt.float32
    fp16 = mybir.dt.float16
    fp8 = mybir.dt.float8e4
    AF = mybir.ActivationFunctionType
    OP = mybir.AluOpType

    KT = F // P              # 16 k-tiles (contraction over features)
    BT = B // P              # 8 batch tiles per core
    MT = MP // P             # 32 padded memory row-tiles
    NCH = MP // 512          # 8 m-chunks of 512 cols for B1
    FC = F // 512            # 4 f-chunks of 512 cols for B2
    HM = MP // 2             # 2048: half the m range (e16/v16 halves)
    thr = 1.0 / M
    LEAD = 3

    nc = bacc.Bacc("TRN2", target_bir_lowering=False, debug=True)
    with tile.TileContext(nc) as tc:
        with tc.tile_pool(name="dram", bufs=1, space="DRAM") as dram:
            xs = dram.tile([B, F], fp32, kind="ExternalInput", uniquify=False, name="xs")
            memory = dram.tile([M, F], fp32, kind="ExternalInput", uniquify=False, name="memory")
            out = dram.tile([B, F], fp32, kind="ExternalOutput", uniquify=False, name="out")

            with tc.tile_pool(name="ps", bufs=8, space="PSUM") as ps, \
                 tc.tile_pool(name="stats", bufs=1) as stats, \
                 tc.tile_pool(name="sml", bufs=4) as sml, \
                 tc.tile_pool(name="l16p", bufs=1) as l16p, \
                 tc.tile_pool(name="e16p", bufs=2) as e16p, \
                 tc.tile_pool(name="v16p", bufs=3) as v16p:

                eps = stats.tile([P, 1], fp32)
                nc.gpsimd.memset(eps[:], 1e-30)
                invz = stats.tile([P, BT], fp32)
                invV = stats.tile([P, BT], fp32)
                l16 = l16p.tile([P, BT, M], fp16)   # fp16 logit store

                vts = [None] * BT    # (vA, vB) tiles pending transpose

                def thr_part(bt):
                    # e = exp(l) in two halves; accums give S = sum_m e
                    eA = e16p.tile([P, HM], fp16, tag="e16", name=f"eA{bt}")
                    SeA = sml.tile([P, 1], fp32, tag="SeA", bufs=2,
                                   name=f"SeA{bt}")
                    nc.scalar.activation(eA[:], l16[:, bt, :HM], AF.Exp,
                                         accum_out=SeA[:])
                    eB = e16p.tile([P, M - HM], fp16, tag="e16",
                                   name=f"eB{bt}")
                    SeB = sml.tile([P, 1], fp32, tag="SeB", bufs=2,
                                   name=f"SeB{bt}")
                    nc.scalar.activation(eB[:], l16[:, bt, HM:], AF.Exp,
                                         accum_out=SeB[:])
                    # u = thr*S - 1;  lnT = ln(1+u) ~ u*(1 - u/2)
                    Se = sml.tile([P, 1], fp32, tag="Se", bufs=2,
                                  name=f"Se{bt}")
                    nc.vector.tensor_tensor(Se[:], SeA[:], SeB[:], op=OP.add)
                    u = sml.tile([P, 1], fp32, tag="u", bufs=2, name=f"u{bt}")
                    nc.vector.tensor_scalar(
                        out=u[:], in0=Se[:], scalar1=thr, scalar2=-1.0,
                        op0=OP.mult, op1=OP.add)
                    h = sml.tile([P, 1], fp32, tag="h", bufs=2, name=f"h{bt}")
                    nc.vector.tensor_scalar(
                        out=h[:], in0=u[:], scalar1=-0.5, scalar2=1.0,
                        op0=OP.mult, op1=OP.add)
                    lnT = sml.tile([P, 1], fp32, tag="lnT", bufs=2,
                                   name=f"lnT{bt}")
                    nc.vector.scalar_tensor_tensor(
                        out=lnT[:], in0=h[:], scalar=1.0, in1=u[:],
                        op0=OP.bypass, op1=OP.mult)
                    # v = (l > lnT) * e, half by half
                    vA = v16p.tile([P, HM], fp16, tag="v16", name=f"vA{bt}")
                    vsA = sml.tile([P, 1], fp32, tag="vsA", bufs=2,
                                   name=f"vsA{bt}")
                    nc.vector.scalar_tensor_tensor(
                        out=vA[:], in0=l16[:, bt, :HM], scalar=lnT[:],
                        in1=eA[:], op0=OP.is_gt, op1=OP.mult,
                        accum_out=vsA[:])
                    vB = v16p.tile([P, HM], fp16, tag="v16", name=f"vB{bt}")
                    if bt < 3:
                        nc.vector.memset(vB[:, M - HM:], 0.0)
                    vsB = sml.tile([P, 1], fp32, tag="vsB", bufs=2,
                                   name=f"vsB{bt}")
                    nc.vector.scalar_tensor_tensor(
                        out=vB[:, :M - HM], in0=l16[:, bt, HM:], scalar=lnT[:],
                        in1=eB[:], op0=OP.is_gt, op1=OP.mult,
                        accum_out=vsB[:])
                    vsum = sml.tile([P, 1], fp32, tag="vsum", bufs=2,
                                    name=f"vsum{bt}")
                    nc.vector.tensor_tensor(vsum[:], vsA[:], vsB[:], op=OP.add)
                    nc.vector.reciprocal(invV[:, bt:bt + 1], vsum[:])
                    vts[bt] = (vA, vB)

                # ---- fused phase A + B1 ----
                with tc.tile_pool(name="xTp", bufs=1) as xTp, \
                     tc.tile_pool(name="mch", bufs=3) as mchp, \
                     tc.tile_pool(name="ain", bufs=4) as ainp, \
                     tc.tile_pool(name="ah16", bufs=2) as ah16p:
                    xT = xTp.tile([P, KT, B], fp16)
                    sqd = xTp.tile([P, F], fp8, name="sqd")

                    zsq8 = stats.tile([P, BT], fp32, name="zsq8")

                    def x_load(bt):
                        xin = ainp.tile([P, F], fp32, tag="ain", bufs=4,
                                        name=f"xin{bt}")
                        nc.sync.dma_start(xin[:], xs[bt * P:(bt + 1) * P, :])
                        return xin

                    def x_comp(bt, xin):
                        nc.vector.scalar_tensor_tensor(
                            out=sqd[:], in0=xin[:], scalar=1.0, in1=xin[:],
                            op0=OP.bypass, op1=OP.mult,
                            accum_out=zsq8[:, bt:bt + 1])
                        xh = ah16p.tile([P, F], fp16, tag="ah", bufs=2,
                                        name=f"xh{bt}")
                        nc.scalar.activation(xh[:], xin[:], AF.Copy)
                        nc.sync.dma_start_transpose(
                            xT[:, :, bt * P:(bt + 1) * P], xh[:])

                    def chunk_load(c):
                        mins = []
                        for j in range(4):
                            mt = c * 4 + j
                            rows = min(P, M - mt * P)
                            min_ = ainp.tile([P, F], fp32, tag="ain", bufs=4,
                                             name=f"min{mt}")
                            if rows < P:
                                nc.vector.memset(min_[:], 0.0)
                                nc.sync.dma_start(
                                    min_[:rows, :],
                                    memory[mt * P:mt * P + rows, :])
                            else:
                                nc.sync.dma_start(
                                    min_[:], memory[mt * P:(mt + 1) * P, :])
                            mins.append(min_)
                        return mins

                    def chunk_comp(c, mins):
                        ct = mchp.tile([P, KT, 512], fp16, tag="mch", bufs=3,
                                       name=f"mch{c}")
                        nsq4 = sml.tile([P, 4], fp32, tag="nsq4", bufs=2,
                                        name=f"nsq4_{c}")
                        for j in range(4):
                            # write-only dump; only the accum row-sum is used
                            nc.vector.scalar_tensor_tensor(
                                out=sqd[:], in0=mins[j][:], scalar=1.0,
                                in1=mins[j][:], op0=OP.bypass, op1=OP.mult,
                                accum_out=nsq4[:, j:j + 1])
                        s4 = sml.tile([P, 4], fp32, tag="s4", bufs=2,
                                      name=f"s4_{c}")
                        nc.scalar.activation(s4[:], nsq4[:], AF.Sqrt,
                                             bias=eps[:])
                        im4 = sml.tile([P, 4], fp32, tag="im4", bufs=2,
                                       name=f"im4_{c}")
                        nc.vector.reciprocal(im4[:], s4[:])
                        for j in range(4):
                            mh = ah16p.tile([P, F], fp16, tag="ah", bufs=2,
                                            name=f"mh{c}_{j}")
                            nc.scalar.activation(mh[:], mins[j][:], AF.Copy,
                                                 scale=im4[:, j:j + 1])
                            nc.sync.dma_start_transpose(
                                ct[:, :, j * P:(j + 1) * P], mh[:])
                        return ct

                    def chunk_prep(c):
                        return chunk_comp(c, chunk_load(c))

                    # prologue: loads queued ahead of compute; x casts for
                    # later bt deferred behind the first chunks.
                    chunks = {}
                    xl = {0: x_load(0)}
                    c0m = chunk_load(0)
                    xl[1] = x_load(1)
                    xl[2] = x_load(2)
                    c1m = chunk_load(1)
                    for bt in range(3, BT):
                        xl[bt] = x_load(bt)
                    x_comp(0, xl[0])
                    chunks[0] = chunk_comp(0, c0m)
                    x_comp(1, xl[1])
                    x_comp(2, xl[2])
                    chunks[1] = chunk_comp(1, c1m)
                    for bt in range(3, BT):
                        x_comp(bt, xl[bt])
                    chunks[2] = chunk_prep(2)
                    s8 = stats.tile([P, BT], fp32, name="s8")
                    nc.scalar.activation(s8[:], zsq8[:], AF.Sqrt, bias=eps[:])
                    nc.vector.reciprocal(invz[:], s8[:])

                    for c in range(NCH):
                        if c + LEAD < NCH:
                            chunks[c + LEAD] = chunk_prep(c + LEAD)
                        cols = min(512, M - c * 512)
                        ct = chunks.pop(c)
                        for bt in range(BT):
                            pt = ps.tile([P, 512], fp32, tag="pb",
                                         name=f"pb{c}_{bt}")
                            for k in range(KT):
                                nc.tensor.matmul(
                                    pt[:, :cols],
                                    lhsT=xT[:, k, bt * P:(bt + 1) * P],
                                    rhs=ct[:, k, :cols],
                                    start=(k == 0), stop=(k == KT - 1))
                            # logit store (fp16)
                            nc.vector.tensor_scalar_mul(
                                l16[:, bt, c * 512:c * 512 + cols],
                                pt[:, :cols], invz[:, bt:bt + 1])
                            if c == NCH - 1:
                                thr_part(bt)

                # ---- v transposes (alias freed xT/ain space) ----
                with tc.tile_pool(name="vTp", bufs=1) as vTp:
                    vT = []
                    for bt in range(BT):
                        vt = vTp.tile([P, MT, P], fp16, tag="vT", bufs=8,
                                      name=f"vT{bt}")
                        vA, vB = vts[bt]
                        nc.sync.dma_start_transpose(vt[:, :MT // 2, :], vA[:])
                        nc.sync.dma_start_transpose(vt[:, MT // 2:, :], vB[:])
                        vT.append(vt)

                    # ---- B2: out = (v/sum v) @ mem ----
                    with tc.tile_pool(name="natp", bufs=6) as natp, \
                         tc.tile_pool(name="evp", bufs=4) as evp:
                        for fc in range(FC):
                            pbs = []
                            for bt in range(BT):
                                pbs.append(ps.tile([P, 512], fp32, tag="pb",
                                                   name=f"pb2_{fc}_{bt}"))
                            for m in range(MT):
                                rows = min(P, M - m * P)
                                nt = natp.tile([P, 512], fp16, tag="nat",
                                               bufs=6, name=f"nat{fc}_{m}")
                                if rows < P:
                                    nc.vector.memset(nt[:], 0.0)
                                    nc.gpsimd.dma_start(
                                        nt[:rows, :],
                                        memory[m * P:m * P + rows,
                                               fc * 512:(fc + 1) * 512])
                                else:
                                    nc.gpsimd.dma_start(
                                        nt[:],
                                        memory[m * P:(m + 1) * P,
                                               fc * 512:(fc + 1) * 512])
                                for bt in range(BT):
                                    nc.tensor.matmul(
                                        pbs[bt][:],
                                        lhsT=vT[bt][:, m, :],
                                        rhs=nt[:],
                                        start=(m == 0), stop=(m == MT - 1))
                            for bt in range(BT):
                                ev = evp.tile([P, 512], fp32, tag="ev",
                                              bufs=4, name=f"ev{fc}_{bt}")
                                nc.scalar.activation(
                                    ev[:], pbs[bt][:], AF.Copy,
                                    scale=invV[:, bt:bt + 1])
                                nc.sync.dma_start(
                                    out[bt * P:(bt + 1) * P,
                                        fc * 512:(fc + 1) * 512], ev[:])
    nc.compile()
    return nc


def _get_nc():
    if "nc" not in _CACHE:
        _CACHE["nc"] = build_nc()
    return _CACHE["nc"]


def kernel(x: np.ndarray, memory: np.ndarray) -> np.ndarray:
    from concourse.bass_utils import run_bass_kernel_spmd

    x = np.ascontiguousarray(x, dtype=np.float32)
    memory = np.ascontiguousarray(memory, dtype=np.float32)
    nc = _get_nc()
    in_maps = [
        {"xs": x[c * B:(c + 1) * B], "memory": memory} for c in range(N_CORES)
    ]
    res = run_bass_kernel_spmd(nc, in_maps, core_ids=list(range(N_CORES)))
    return np.concatenate([res.results[c]["out"] for c in range(N_CORES)], axis=0)
